# revision 39
# baseline (speedup 1.0000x reference)
"""GCN (2-layer + linear head) on 8 Trainium2 NeuronCores.

Math: with Ahat = D^-1/2 (A+I) D^-1/2 and dinv = deg^-1/2,
  h1 = relu((Ahat x) W1 + b1)
  h2 = relu((Ahat h1) W2 + b2)        [Ahat h = dinv * (A+I)(dinv * h)]
  out = h2 Wl + bl

Sharding: nodes row-sharded 6250/core (dst side); edges bucketed by dst
window (128 nodes); aggregation = one-hot selection matmuls on PE over
bf16 messages.

Structure (vs the serial baseline):
  - L1 messages are HOST-materialized: the edge stream of dinv-scaled x
    rows ships as an input tensor, so L1 needs no device-side gather at
    all (plain sequential DMA loads).  Self-loops ride along as one
    extra identity batch per window.
  - The inter-layer AllGather of h1*dinv is split into 5 row chunks
    (11/11/10/9/8 windows, chunk boundary aligned at gathered row
    32768) issued as soon as their L1 windows complete -> comm overlaps
    L1 compute and L2 processing.
  - L2 messages use dma_gather with indices REBASED per AllGather chunk
    (in_ap = that chunk's gout slice), so every index fits int16 and
    each gather depends on exactly one AllGather chunk.  Gathers are
    spread round-robin over 4 SWDGE queues (~3x descriptor rate).
  - L2 runs in two passes (chunks {0,1,2} then {3,4}) with a per-window
    bf16 SBUF accumulator carried between passes via an identity-matmul
    PSUM preload; fp8 batch matmuls are paired with perf_mode=DoubleRow;
    window tails (dense matmuls + head) run interleaved with the last
    pass.  L2 self-loops are an identity matmul of the core's own h1
    rows (kept bf16).  Scales/relus/copies run on the Scalar engine and
    L1 stream loads on the GPSIMD SWDGE queue, keeping the Vector
    engine quiet (its SBUF port is shared with the gather engine) and
    letting loads prefetch ahead of compute.
  - The AllGather payload and gathered L2 messages are fp8 (e4m3):
    halves both the collective wire time and the gather bandwidth,
    which are the two dominant costs.  Everything else stays bf16/f32.
  - gin2/gout are per-chunk DRAM tiles so Tile's dependency tracking
    ties each AllGather to exactly its windows (no false WAR), and the
    dense W2 stage computes h2 directly in transposed form.
"""
from contextlib import ExitStack

import numpy as np
import ml_dtypes

N = 50000
E = 800000
IN, H, OUT = 256, 512, 64
NCORES = 8
S_OWN = N // NCORES            # 6250 rows per core
P = 128
NWIN = (S_OWN + P - 1) // P    # 49 windows per core
CHUNK_B = 8                    # batches per gather/stream chunk (1024 rows)
NIDX = CHUNK_B * P
SUP = 2                        # L1 stream chunks per dma (8KB/partition)

# AllGather chunking (windows per chunk; boundary after 32 windows -> row
# 32768 in gout, keeping chunks 0-2 entirely below the int16 split)
WCH = [11, 11, 10, 9, 8]
CUMW = [0, 11, 22, 32, 41, 49]
CUMR = [c * P for c in CUMW[:-1]]            # [0, 1408, 2816, 4096, 5248]
RRE = [1408, 1408, 1280, 1152, S_OWN - 5248]  # real rows/chunk (last=1002)
GBASE = [8 * c for c in CUMR]                # [0, 11264, 22528, 32768, 41984]
NG = len(WCH)

BF16 = ml_dtypes.bfloat16


# ---------------------------------------------------------------- host prep

def _wrap_idx(idx, nch):
    """[nch*NIDX] -> [P, nch, NIDX//16] int16 dma_gather index layout."""
    idxr = idx.astype(np.int16).reshape(nch, NIDX)
    j = np.arange(NIDX)
    wrap = np.zeros((nch, 16, NIDX // 16), dtype=np.int16)
    wrap[:, j % 16, j // 16] = idxr
    rep = np.tile(wrap, (1, 8, 1))           # [nch, 128, NIDX//16]
    return np.ascontiguousarray(rep.transpose(1, 0, 2))


def _prep(edge_index):
    """Host-side plan. Returns per-core tables + batch schedules."""
    src = edge_index[0].astype(np.int64)
    dst = edge_index[1].astype(np.int64)

    deg = np.bincount(dst, minlength=N).astype(np.float32) + 1.0  # + self loop
    dinv = 1.0 / np.sqrt(deg)

    core = dst // S_OWN
    win = (dst % S_OWN) >> 7
    dloc = (dst % S_OWN) & 127                    # dst slot within window

    # ---- L1: batches per (core, win), self batch prepended per window
    order1 = np.lexsort((src, win, core))
    cnt1 = np.bincount(core * NWIN + win, minlength=NCORES * NWIN).reshape(
        NCORES, NWIN)
    bpw1 = -(-cnt1.max(axis=0) // P)              # edge batches per window
    nbat1 = int(bpw1.sum()) + NWIN                # + one self batch per window
    nch1 = -(-nbat1 // (CHUNK_B * SUP)) * SUP     # pad to super-chunk multiple
    # per-core edge stream source ids + dst slots, window-major with padding
    src1 = np.zeros((NCORES, nch1 * NIDX), dtype=np.int64)
    dst1 = np.full((NCORES, nch1 * NIDX), -1.0, dtype=np.float32)
    sched1 = [[] for _ in range(NWIN)]            # [(chunk, slot)] per window
    off1 = np.concatenate([[0], np.cumsum(bpw1 + 1)]) * P
    for w in range(NWIN):
        t = off1[w]
        nrow = min(P, S_OWN - w * P)
        b0 = t // P
        for b in range(b0, b0 + 1 + int(bpw1[w])):
            sched1[w].append((b // CHUNK_B, b % CHUNK_B))
    seg_off = np.concatenate([[0], np.cumsum(cnt1.ravel())])
    s1 = src[order1]
    d1 = dloc[order1]
    for k in range(NCORES):
        for w in range(NWIN):
            t = off1[w]
            nrow = min(P, S_OWN - w * P)
            # self batch: own rows, dst = iota
            src1[k, t : t + nrow] = k * S_OWN + w * P + np.arange(nrow)
            dst1[k, t : t + nrow] = np.arange(nrow)
            o = seg_off[k * NWIN + w]
            n = cnt1[k, w]
            src1[k, t + P : t + P + n] = s1[o : o + n]
            dst1[k, t + P : t + P + n] = d1[o : o + n]

    # ---- L2: bucket edges by (chunk of src's gathered row, win)
    ksrc = src // S_OWN
    lsrc = src % S_OWN
    g_of = np.searchsorted(np.asarray(CUMR + [S_OWN]), lsrc, side="right") - 1
    grow = np.zeros(E, dtype=np.int64)            # index within chunk table
    for g in range(NG):
        m = g_of == g
        grow[m] = ksrc[m] * RRE[g] + (lsrc[m] - CUMR[g])

    order2 = np.lexsort((grow, win, g_of, core))
    key2 = (core * NG + g_of) * NWIN + win
    cnt2 = np.bincount(key2, minlength=NCORES * NG * NWIN).reshape(
        NCORES, NG, NWIN)
    bpw2 = -(-cnt2.max(axis=0) // P)              # [NG, NWIN]
    nbat2 = bpw2.sum(axis=1)                      # batches per chunk table
    nch2 = [-(-int(nb) // CHUNK_B) for nb in nbat2]
    idx2 = [np.zeros((NCORES, nch2[g] * NIDX), dtype=np.int64) for g in range(NG)]
    dst2 = [np.full((NCORES, nch2[g] * NIDX), -1.0, dtype=np.float32)
            for g in range(NG)]
    sched2 = [[[] for _ in range(NWIN)] for _ in range(NG)]
    g2 = grow[order2]
    d2 = dloc[order2]
    seg_off2 = np.concatenate([[0], np.cumsum(cnt2.ravel())])
    for g in range(NG):
        off = np.concatenate([[0], np.cumsum(bpw2[g])]) * P
        for w in range(NWIN):
            for b in range(off[w] // P, off[w] // P + int(bpw2[g, w])):
                sched2[g][w].append((b // CHUNK_B, b % CHUNK_B))
        for k in range(NCORES):
            for w in range(NWIN):
                o = seg_off2[(k * NG + g) * NWIN + w]
                n = cnt2[k, g, w]
                t = off[w]
                idx2[g][k, t : t + n] = g2[o : o + n]
                dst2[g][k, t : t + n] = d2[o : o + n]

    # device layouts
    idx2_t = [np.stack([_wrap_idx(idx2[g][k], nch2[g]) for k in range(NCORES)])
              for g in range(NG)]

    def dst_layout(d, nch):
        # [cores, nch*NIDX] -> [cores, P, nch, CHUNK_B] (edge p of batch -> part p)
        return np.ascontiguousarray(
            d.reshape(NCORES, nch, CHUNK_B, P).transpose(0, 3, 1, 2))

    dst1_t = dst_layout(dst1, nch1)
    dst2_t = [dst_layout(dst2[g], nch2[g]) for g in range(NG)]

    return dict(deg=deg, dinv=dinv, src1=src1, nch1=nch1, sched1=sched1,
                dst1_t=dst1_t, idx2_t=idx2_t, dst2_t=dst2_t, nch2=nch2,
                sched2=sched2)


# ---------------------------------------------------------------- device

def _build_nc(nch1, nch2, sched1, sched2):
    from concourse import bacc, mybir
    import concourse.tile as tile
    from concourse.masks import make_identity

    f32 = mybir.dt.float32
    bf = mybir.dt.bfloat16
    f8 = mybir.dt.float8e4
    i16 = mybir.dt.int16

    nc = bacc.Bacc("TRN2", target_bir_lowering=False, debug=False,
                   num_devices=NCORES, num_swdge_queues=4)

    ms_d = nc.dram_tensor("ms", [P, nch1 * CHUNK_B * IN], bf, kind="ExternalInput")
    dst1_d = nc.dram_tensor("dst1", [P, nch1 * CHUNK_B], bf, kind="ExternalInput")
    idx2_d = [nc.dram_tensor(f"idx2_{g}", [P, nch2[g] * (NIDX // 16)], i16,
                             kind="ExternalInput") for g in range(NG)]
    dst2_d = [nc.dram_tensor(f"dst2_{g}", [P, nch2[g] * CHUNK_B], bf,
                             kind="ExternalInput") for g in range(NG)]
    dinvo_d = nc.dram_tensor("dinvo", [P, NWIN], f32, kind="ExternalInput")
    w1_d = nc.dram_tensor("w1", [P, IN // P, H], bf, kind="ExternalInput")
    w2_d = nc.dram_tensor("w2", [P, H // P, H], bf, kind="ExternalInput")
    wl_d = nc.dram_tensor("wl", [P, H // P, OUT], bf, kind="ExternalInput")
    b1_d = nc.dram_tensor("b1", [1, H], bf, kind="ExternalInput")
    b2_d = nc.dram_tensor("b2", [1, H], bf, kind="ExternalInput")
    bl_d = nc.dram_tensor("bl", [1, OUT], bf, kind="ExternalInput")
    out_d = nc.dram_tensor("out", [S_OWN, OUT], f32, kind="ExternalOutput")

    with tile.TileContext(nc) as tc, ExitStack() as ctx:
        cpool = ctx.enter_context(tc.tile_pool(name="const", bufs=1))
        dram = ctx.enter_context(tc.tile_pool(name="dram", bufs=1, space="DRAM"))
        l1pool = ctx.enter_context(tc.tile_pool(name="l1m", bufs=6))
        hopool = ctx.enter_context(tc.tile_pool(name="h1o", bufs=3))
        mpool = ctx.enter_context(tc.tile_pool(name="msg", bufs=10))
        spool = ctx.enter_context(tc.tile_pool(name="sel", bufs=5))
        ypool = ctx.enter_context(tc.tile_pool(name="ys", bufs=3))
        hpool = ctx.enter_context(tc.tile_pool(name="dense", bufs=3))
        psA = ctx.enter_context(tc.tile_pool(name="psA", bufs=2, space="PSUM"))
        psB = ctx.enter_context(tc.tile_pool(name="psB", bufs=2, space="PSUM"))
        psT = ctx.enter_context(tc.tile_pool(name="psT", bufs=2, space="PSUM"))

        # ---- constants
        iota_i = cpool.tile([P, P], mybir.dt.int32)
        iota_b = cpool.tile([P, CHUNK_B, P], bf)
        nc.gpsimd.iota(iota_i[:], pattern=[[1, P]], base=0, channel_multiplier=0)
        for bc in range(CHUNK_B):
            nc.vector.tensor_copy(out=iota_b[:, bc], in_=iota_i[:])
        ident = cpool.tile([P, P], bf)
        make_identity(nc, ident[:])
        ones_t = cpool.tile([1, P], bf)
        nc.vector.memset(ones_t[:], 1.0)

        dinvo = cpool.tile([P, NWIN], f32)
        nc.sync.dma_start(out=dinvo[:], in_=dinvo_d[:])

        dst1_t = cpool.tile([P, nch1, CHUNK_B], bf)
        nc.sync.dma_start(out=dst1_t[:], in_=dst1_d[:].rearrange(
            "p (c b) -> p c b", b=CHUNK_B))
        idx2_t = []
        dst2_t = []
        for g in range(NG):
            it = cpool.tile([P, nch2[g], NIDX // 16], mybir.dt.int16,
                            name=f"idx2t{g}")
            idx2_t.append(it)
            dt_ = cpool.tile([P, nch2[g], CHUNK_B], bf, name=f"dst2t{g}")
            dst2_t.append(dt_)

        def load_tables(g):
            # deferred: staggered through L1 so the startup burst does not
            # contend with the L1 stream loads
            nc.sync.dma_start(out=idx2_t[g][:], in_=idx2_d[g][:].rearrange(
                "p (c j) -> p c j", j=NIDX // 16))
            nc.sync.dma_start(out=dst2_t[g][:], in_=dst2_d[g][:].rearrange(
                "p (c b) -> p c b", b=CHUNK_B))

        w1_t = cpool.tile([P, IN // P, H], bf)
        w2_t = cpool.tile([P, H // P, H], bf)
        wl_t = cpool.tile([P, H // P, OUT], bf)
        b1_t = cpool.tile([1, H], bf)
        b2_t = cpool.tile([1, H], bf)
        bl_t = cpool.tile([1, OUT], bf)
        for t, d in ((w1_t, w1_d), (w2_t, w2_d), (wl_t, wl_d),
                     (b1_t, b1_d), (b2_t, b2_d), (bl_t, bl_d)):
            nc.sync.dma_start(out=t[:], in_=d[:])

        # accumulator for pipelined L2 (bf16; one column block per window)
        acc2 = cpool.tile([P, NWIN, H], bf)

        # ---- DRAM intermediates (per AllGather chunk so deps stay exact:
        # no false WAR between later L1 windows and an in-flight AllGather)
        gin2b = [dram.tile([RRE[g], H], bf, name=f"gin2b{g}")
                 for g in range(NG)]
        gin2f = [dram.tile([RRE[g], H], f8, name=f"gin2f{g}")
                 for g in range(NG)]
        gout = [dram.tile([8 * RRE[g], H], f8, addr_space="Shared",
                          name=f"gout{g}") for g in range(NG)]

        # ---- L1: host-materialized stream, window-major.  Loads are
        # super-chunks of SUP gather-chunks: partition-major layout gives
        # 16KB contiguous per partition per dma (big descriptors).
        loaded1s = {}
        loaded1 = {}
        SCB = SUP * CHUNK_B * IN

        def ensure1(c):
            if c in loaded1:
                return loaded1[c]
            s = c // SUP
            if s not in loaded1s:
                mts = l1pool.tile([P, SUP, CHUNK_B, IN], bf, tag="m1")
                # SWDGE: the gpsimd queue carries no per-window compute in L1,
                # so these loads issue as far ahead as the slots allow
                nc.gpsimd.dma_start(
                    out=mts[:], in_=ms_d[:, s * SCB : (s + 1) * SCB]
                    .rearrange("p (u b f) -> p u b f", u=SUP, b=CHUNK_B))
                loaded1s[s] = mts
            mt = loaded1s[s][:, c % SUP]
            st = spool.tile([P, CHUNK_B, P], bf, tag="sel")
            nc.vector.tensor_tensor(
                out=st[:], in0=iota_b[:],
                in1=dst1_t[:, c].to_broadcast([P, CHUNK_B, P]),
                op=mybir.AluOpType.is_equal)
            loaded1[c] = (mt, st)
            return mt, st

        # ---- L2: chunk-major pipelined aggregation
        rrq = [0]

        def ensure2(g, c, loaded2):
            if (g, c) in loaded2:
                return loaded2[(g, c)]
            mt = mpool.tile([P, CHUNK_B, H], f8, tag="m2")
            nc.gpsimd.dma_gather(
                out_ap=mt[:], in_ap=gout[g][:],
                idxs_ap=idx2_t[g][:, c], num_idxs=NIDX, num_idxs_reg=NIDX,
                elem_size=H, queue_num=rrq[0])
            rrq[0] = (rrq[0] + 1) % 4
            st = spool.tile([P, CHUNK_B, P], f8, tag="sel2")
            nc.vector.tensor_tensor(
                out=st[:], in0=iota_b[:],
                in1=dst2_t[g][:, c].to_broadcast([P, CHUNK_B, P]),
                op=mybir.AluOpType.is_equal)
            loaded2[(g, c)] = (mt, st)
            return mt, st

        def tail2(w, src):
            nrow = min(P, S_OWN - w * P)
            ys2 = ypool.tile([P, H], bf, tag="ys2")
            nc.scalar.mul(ys2[:], src, dinvo[:, w : w + 1])
            aggT = hpool.tile([P, H // P, P], bf, tag="aggT2")
            for f in range(H // P):
                tp = psT.tile([P, P], bf, tag="tp", bufs=1)
                nc.tensor.transpose(tp[:], ys2[:, f * P : (f + 1) * P], ident[:])
                nc.scalar.copy(out=aggT[:, f], in_=tp[:])
            # h2T computed directly in transposed form: no second transpose
            ph2T = psB.tile([P, H], f32, tag="mm")
            for fh in range(H // P):
                o = ph2T[:, fh * P : (fh + 1) * P]
                for fc in range(H // P):
                    nc.tensor.matmul(out=o, rhs=aggT[:, fc],
                                     lhsT=w2_t[:, fc, fh * P : (fh + 1) * P],
                                     start=(fc == 0), stop=False)
                nc.tensor.matmul(out=o, lhsT=b2_t[:, fh * P : (fh + 1) * P],
                                 rhs=ones_t[:], start=False, stop=True)
            h2T = hpool.tile([P, H // P, P], bf, tag="h2T")
            nc.scalar.activation(h2T[:],
                                 ph2T[:].rearrange("p (f j) -> p f j", j=P),
                                 mybir.ActivationFunctionType.Relu)
            ph3 = psB.tile([P, OUT], f32, tag="mm")
            for f in range(H // P):
                nc.tensor.matmul(out=ph3[:], lhsT=h2T[:, f], rhs=wl_t[:, f],
                                 start=(f == 0), stop=False)
            nc.tensor.matmul(out=ph3[:], lhsT=ones_t[:], rhs=bl_t[:],
                             start=False, stop=True)
            ot = hpool.tile([P, OUT], f32, tag="ot")
            nc.scalar.copy(out=ot[:], in_=ph3[:])
            nc.sync.dma_start(out=out_d[w * P : w * P + nrow], in_=ot[:nrow])


        loaded2 = {}
        dr = mybir.MatmulPerfMode.DoubleRow

        def run_batches(pp, items):
            i = 0
            n = len(items)
            while i < n:
                g, c, bc = items[i]
                mt, st = ensure2(g, c, loaded2)
                if (i + 1 < n and items[i + 1] == (g, c, bc + 1)):
                    # fp8 DoubleRow: two 128-edge batches per instruction
                    nc.tensor.matmul(out=pp[:], lhsT=st[:, bc : bc + 2],
                                     rhs=mt[:, bc : bc + 2], start=False,
                                     stop=(i + 2 == n), perf_mode=dr)
                    i += 2
                else:
                    nc.tensor.matmul(out=pp[:], lhsT=st[:, bc],
                                     rhs=mt[:, bc], start=False,
                                     stop=(i + 1 == n))
                    i += 1

        def pass0_window(w):
            # chunks {0,1,2} + self-loop (identity matmul of own bf16 h1 rows)
            items = [(g, c, bc) for g in (0, 1, 2) for (c, bc) in sched2[g][w]]
            pp = psA.tile([P, H], f32, tag="acc2")
            h1own = hopool.tile([P, H], bf, tag="h1own")
            nrow = min(P, S_OWN - w * P)
            gw = next(gg for gg in range(NG) if w < CUMW[gg + 1])
            wrow = (w - CUMW[gw]) * P
            if nrow < P:
                nc.vector.memset(h1own[:], 0.0)
            nc.sync.dma_start(out=h1own[:nrow],
                              in_=gin2b[gw][wrow : wrow + nrow])
            nc.tensor.matmul(out=pp[:], lhsT=ident[:], rhs=h1own[:],
                             start=True, stop=(len(items) == 0))
            run_batches(pp, items)
            nc.scalar.copy(out=acc2[:, w], in_=pp[:])

        def pass1_window(w):
            # chunks {3,4}: resume from acc2 via PE preload, then tail
            items = [(g, c, bc) for g in (3, 4) for (c, bc) in sched2[g][w]]
            pp = psA.tile([P, H], f32, tag="acc2")
            nc.tensor.matmul(out=pp[:], lhsT=ident[:], rhs=acc2[:, w],
                             start=True, stop=(len(items) == 0))
            run_batches(pp, items)
            tail2(w, pp[:])

        TBL_AT = {2: 0, 12: 1, 23: 2, 33: 3, 42: 4}
        for w in range(NWIN):
            if w in TBL_AT:
                load_tables(TBL_AT[w])
            nrow = min(P, S_OWN - w * P)
            acc = psA.tile([P, IN], f32, tag="acc1", bufs=3)
            nbat = len(sched1[w])
            for i, (c, bc) in enumerate(sched1[w]):
                mt, st = ensure1(c)
                nc.tensor.matmul(out=acc[:], lhsT=st[:, bc], rhs=mt[:, bc],
                                 start=(i == 0), stop=(i == nbat - 1))
            ys = ypool.tile([P, IN], bf, tag="ys1")
            nc.scalar.mul(ys[:], acc[:], dinvo[:, w : w + 1])
            aggT = hpool.tile([P, IN // P, P], bf, tag="aggT1")
            for f in range(IN // P):
                tp = psT.tile([P, P], bf, tag="tp", bufs=1)
                nc.tensor.transpose(tp[:], ys[:, f * P : (f + 1) * P], ident[:])
                nc.scalar.copy(out=aggT[:, f], in_=tp[:])
            ph = psB.tile([P, H], f32, tag="mm")
            for f in range(IN // P):
                nc.tensor.matmul(out=ph[:], lhsT=aggT[:, f], rhs=w1_t[:, f],
                                 start=(f == 0), stop=False)
            nc.tensor.matmul(out=ph[:], lhsT=ones_t[:], rhs=b1_t[:],
                             start=False, stop=True)
            g2 = hpool.tile([P, H], bf, tag="g2")
            nc.scalar.activation(g2[:], ph[:], mybir.ActivationFunctionType.Relu,
                                 scale=dinvo[:, w : w + 1])
            g2f = hpool.tile([P, H], f8, tag="g2f")
            nc.scalar.activation(g2f[:], ph[:], mybir.ActivationFunctionType.Relu,
                                 scale=dinvo[:, w : w + 1])
            gw = next(g for g in range(NG) if w < CUMW[g + 1])
            wrow = (w - CUMW[gw]) * P
            nc.sync.dma_start(out=gin2b[gw][wrow : wrow + nrow], in_=g2[:nrow])
            nc.sync.dma_start(out=gin2f[gw][wrow : wrow + nrow], in_=g2f[:nrow])

            for g in range(NG):
                if w == CUMW[g + 1] - 1:
                    nc.gpsimd.collective_compute(
                        "AllGather", mybir.AluOpType.bypass,
                        replica_groups=[list(range(NCORES))],
                        ins=[gin2f[g][:]],
                        outs=[gout[g][:]])
                    if g == 3:
                        # fill the PE stall while AG_3/AG_4 drain: process
                        # L2 pass-0 for windows whose inputs are all ready
                        for wp in range(CUMW[3]):
                            pass0_window(wp)

        # remaining L2 work: pass-0 for late windows, then pass-1 + tails
        for w in range(CUMW[3], NWIN):
            pass0_window(w)
        for w in range(NWIN):
            pass1_window(w)

    nc.compile()
    return nc


_CACHE = {}


def _make_in_maps(inputs, prep):
    x = np.asarray(inputs["x"], dtype=np.float32)
    W1 = np.asarray(inputs["W1"], dtype=np.float32)
    b1 = np.asarray(inputs["b1"], dtype=np.float32)
    W2 = np.asarray(inputs["W2"], dtype=np.float32)
    b2 = np.asarray(inputs["b2"], dtype=np.float32)
    Wl = np.asarray(inputs["Wl"], dtype=np.float32)
    bl = np.asarray(inputs["bl"], dtype=np.float32)

    dinv = prep["dinv"]
    xs = (x * dinv[:, None]).astype(BF16)         # pre-scaled source features

    w1b = W1.reshape(IN // P, P, H).transpose(1, 0, 2).astype(BF16)
    w2b = W2.reshape(H // P, P, H).transpose(1, 0, 2).astype(BF16)
    wlb = Wl.reshape(H // P, P, OUT).transpose(1, 0, 2).astype(BF16)

    nch1 = prep["nch1"]
    in_maps = []
    for k in range(NCORES):
        # L1 message stream, partition-major: [P, nch1*CHUNK_B*IN]; edge p
        # of batch (c,bc) lands at [p, (c*CHUNK_B+bc)*IN :]
        ms = xs[prep["src1"][k]]                  # [nch1*NIDX, IN]
        ms = ms.reshape(nch1, CHUNK_B, P, IN).transpose(2, 0, 1, 3)
        ms = np.ascontiguousarray(ms).reshape(P, nch1 * CHUNK_B * IN)
        dv = np.ones(NWIN * P, dtype=np.float32)
        dv[:S_OWN] = dinv[k * S_OWN : (k + 1) * S_OWN]
        dv = np.ascontiguousarray(dv.reshape(NWIN, P).T)
        im = {
            "ms": ms,
            "dst1": prep["dst1_t"][k].reshape(P, -1).astype(BF16),
            "dinvo": dv,
            "w1": w1b, "w2": w2b, "wl": wlb,
            "b1": b1.reshape(1, H).astype(BF16),
            "b2": b2.reshape(1, H).astype(BF16),
            "bl": bl.reshape(1, OUT).astype(BF16),
        }
        for g in range(NG):
            im[f"idx2_{g}"] = prep["idx2_t"][g][k].reshape(P, -1)
            im[f"dst2_{g}"] = prep["dst2_t"][g][k].reshape(P, -1).astype(BF16)
        in_maps.append(im)
    return in_maps


def kernel(**inputs):
    from concourse.bass_utils import run_bass_kernel_spmd

    edge_index = np.asarray(inputs["edge_index"])
    prep = _prep(edge_index)
    key = (prep["nch1"], tuple(prep["nch2"]))
    if key not in _CACHE:
        _CACHE[key] = _build_nc(prep["nch1"], prep["nch2"],
                                prep["sched1"], prep["sched2"])
    nc = _CACHE[key]
    global _LAST_NC
    _LAST_NC = nc.m
    in_maps = _make_in_maps(inputs, prep)

    res = run_bass_kernel_spmd(nc, in_maps, core_ids=list(range(NCORES)))
    out = np.concatenate([res.results[k]["out"] for k in range(NCORES)], axis=0)
    return out


# revision 41
# speedup vs baseline: 1.0749x; 1.0749x over previous
"""GCN (2-layer + linear head) on 8 Trainium2 NeuronCores.

Math: with Ahat = D^-1/2 (A+I) D^-1/2 and dinv = deg^-1/2,
  h1 = relu((Ahat x) W1 + b1)
  h2 = relu((Ahat h1) W2 + b2)        [Ahat h = dinv * (A+I)(dinv * h)]
  out = h2 Wl + bl

Sharding: nodes row-sharded 6250/core (dst side); edges bucketed by dst
window (128 nodes); aggregation = one-hot selection matmuls on PE over
bf16 messages.

Structure (vs the serial baseline):
  - L1 messages are HOST-materialized: the edge stream of dinv-scaled x
    rows ships as an input tensor, so L1 needs no device-side gather at
    all (plain sequential DMA loads).  Self-loops ride along as one
    extra identity batch per window.
  - The inter-layer AllGather of h1*dinv is split into 5 row chunks
    (11/11/10/9/8 windows, chunk boundary aligned at gathered row
    32768) issued as soon as their L1 windows complete -> comm overlaps
    L1 compute and L2 processing.
  - L2 messages use dma_gather with indices REBASED per AllGather chunk
    (in_ap = that chunk's gout slice), so every index fits int16 and
    each gather depends on exactly one AllGather chunk.  Gathers are
    spread round-robin over 4 SWDGE queues (~3x descriptor rate).
  - L2 runs in two passes (chunks {0,1,2} then {3,4}) with a per-window
    bf16 SBUF accumulator carried between passes via an identity-matmul
    PSUM preload; fp8 batch matmuls are paired with perf_mode=DoubleRow;
    window tails (dense matmuls + head) run interleaved with the last
    pass.  L2 self-loops are an identity matmul of the core's own h1
    rows (kept bf16).  Scales/relus/copies run on the Scalar engine and
    L1 stream loads on the GPSIMD SWDGE queue, keeping the Vector
    engine quiet (its SBUF port is shared with the gather engine) and
    letting loads prefetch ahead of compute.
  - The AllGather payload and gathered L2 messages are fp8 (e4m3):
    halves both the collective wire time and the gather bandwidth,
    which are the two dominant costs.  Everything else stays bf16/f32.
  - gin2/gout are per-chunk DRAM tiles so Tile's dependency tracking
    ties each AllGather to exactly its windows (no false WAR), and the
    dense W2 stage computes h2 directly in transposed form.
"""
from contextlib import ExitStack

import numpy as np
import ml_dtypes

N = 50000
E = 800000
IN, H, OUT = 256, 512, 64
NCORES = 8
S_OWN = N // NCORES            # 6250 rows per core
P = 128
NWIN = (S_OWN + P - 1) // P    # 49 windows per core
CHUNK_B = 8                    # batches per gather/stream chunk (1024 rows)
NIDX = CHUNK_B * P
SUP = 2                        # L1 stream chunks per dma (8KB/partition)

# AllGather chunking (windows per chunk; boundary after 32 windows -> row
# 32768 in gout, keeping chunks 0-2 entirely below the int16 split)
WCH = [13, 13, 13, 10]
CUMW = [0, 13, 26, 39, 49]
CUMR = [c * P for c in CUMW[:-1]]            # [0, 1408, 2816, 4096, 5248]
RRE = [CUMR[i + 1] - CUMR[i] for i in range(len(CUMR) - 1)] + [S_OWN - CUMR[-1]]
GBASE = [8 * c for c in CUMR]                # [0, 11264, 22528, 32768, 41984]
NG = len(WCH)

BF16 = ml_dtypes.bfloat16


# ---------------------------------------------------------------- host prep

def _wrap_idx(idx, nch):
    """[nch*NIDX] -> [P, nch, NIDX//16] int16 dma_gather index layout."""
    idxr = idx.astype(np.int16).reshape(nch, NIDX)
    j = np.arange(NIDX)
    wrap = np.zeros((nch, 16, NIDX // 16), dtype=np.int16)
    wrap[:, j % 16, j // 16] = idxr
    rep = np.tile(wrap, (1, 8, 1))           # [nch, 128, NIDX//16]
    return np.ascontiguousarray(rep.transpose(1, 0, 2))


def _prep(edge_index):
    """Host-side plan. Returns per-core tables + batch schedules."""
    src = edge_index[0].astype(np.int64)
    dst = edge_index[1].astype(np.int64)

    deg = np.bincount(dst, minlength=N).astype(np.float32) + 1.0  # + self loop
    dinv = 1.0 / np.sqrt(deg)

    core = dst // S_OWN
    win = (dst % S_OWN) >> 7
    dloc = (dst % S_OWN) & 127                    # dst slot within window

    # ---- L1: batches per (core, win), self batch prepended per window
    order1 = np.lexsort((src, win, core))
    cnt1 = np.bincount(core * NWIN + win, minlength=NCORES * NWIN).reshape(
        NCORES, NWIN)
    bpw1 = -(-cnt1.max(axis=0) // P)              # edge batches per window
    nbat1 = int(bpw1.sum()) + NWIN                # + one self batch per window
    nch1 = -(-nbat1 // (CHUNK_B * SUP)) * SUP     # pad to super-chunk multiple
    # per-core edge stream source ids + dst slots, window-major with padding
    src1 = np.zeros((NCORES, nch1 * NIDX), dtype=np.int64)
    dst1 = np.full((NCORES, nch1 * NIDX), -1.0, dtype=np.float32)
    sched1 = [[] for _ in range(NWIN)]            # [(chunk, slot)] per window
    off1 = np.concatenate([[0], np.cumsum(bpw1 + 1)]) * P
    for w in range(NWIN):
        t = off1[w]
        nrow = min(P, S_OWN - w * P)
        b0 = t // P
        for b in range(b0, b0 + 1 + int(bpw1[w])):
            sched1[w].append((b // CHUNK_B, b % CHUNK_B))
    seg_off = np.concatenate([[0], np.cumsum(cnt1.ravel())])
    s1 = src[order1]
    d1 = dloc[order1]
    for k in range(NCORES):
        for w in range(NWIN):
            t = off1[w]
            nrow = min(P, S_OWN - w * P)
            # self batch: own rows, dst = iota
            src1[k, t : t + nrow] = k * S_OWN + w * P + np.arange(nrow)
            dst1[k, t : t + nrow] = np.arange(nrow)
            o = seg_off[k * NWIN + w]
            n = cnt1[k, w]
            src1[k, t + P : t + P + n] = s1[o : o + n]
            dst1[k, t + P : t + P + n] = d1[o : o + n]

    # ---- L2: bucket edges by (chunk of src's gathered row, win)
    ksrc = src // S_OWN
    lsrc = src % S_OWN
    g_of = np.searchsorted(np.asarray(CUMR + [S_OWN]), lsrc, side="right") - 1
    grow = np.zeros(E, dtype=np.int64)            # index within chunk table
    for g in range(NG):
        m = g_of == g
        grow[m] = ksrc[m] * RRE[g] + (lsrc[m] - CUMR[g])

    order2 = np.lexsort((grow, win, g_of, core))
    key2 = (core * NG + g_of) * NWIN + win
    cnt2 = np.bincount(key2, minlength=NCORES * NG * NWIN).reshape(
        NCORES, NG, NWIN)
    bpw2 = -(-cnt2.max(axis=0) // P)              # [NG, NWIN]
    nbat2 = bpw2.sum(axis=1)                      # batches per chunk table
    nch2 = [-(-int(nb) // CHUNK_B) for nb in nbat2]
    idx2 = [np.zeros((NCORES, nch2[g] * NIDX), dtype=np.int64) for g in range(NG)]
    dst2 = [np.full((NCORES, nch2[g] * NIDX), -1.0, dtype=np.float32)
            for g in range(NG)]
    sched2 = [[[] for _ in range(NWIN)] for _ in range(NG)]
    g2 = grow[order2]
    d2 = dloc[order2]
    seg_off2 = np.concatenate([[0], np.cumsum(cnt2.ravel())])
    for g in range(NG):
        off = np.concatenate([[0], np.cumsum(bpw2[g])]) * P
        for w in range(NWIN):
            for b in range(off[w] // P, off[w] // P + int(bpw2[g, w])):
                sched2[g][w].append((b // CHUNK_B, b % CHUNK_B))
        for k in range(NCORES):
            for w in range(NWIN):
                o = seg_off2[(k * NG + g) * NWIN + w]
                n = cnt2[k, g, w]
                t = off[w]
                idx2[g][k, t : t + n] = g2[o : o + n]
                dst2[g][k, t : t + n] = d2[o : o + n]

    # device layouts
    idx2_t = [np.stack([_wrap_idx(idx2[g][k], nch2[g]) for k in range(NCORES)])
              for g in range(NG)]

    def dst_layout(d, nch):
        # [cores, nch*NIDX] -> [cores, P, nch, CHUNK_B] (edge p of batch -> part p)
        return np.ascontiguousarray(
            d.reshape(NCORES, nch, CHUNK_B, P).transpose(0, 3, 1, 2))

    dst1_t = dst_layout(dst1, nch1)
    dst2_t = [dst_layout(dst2[g], nch2[g]) for g in range(NG)]

    return dict(deg=deg, dinv=dinv, src1=src1, nch1=nch1, sched1=sched1,
                dst1_t=dst1_t, idx2_t=idx2_t, dst2_t=dst2_t, nch2=nch2,
                sched2=sched2)


# ---------------------------------------------------------------- device

def _build_nc(nch1, nch2, sched1, sched2):
    from concourse import bacc, mybir
    import concourse.tile as tile
    from concourse.masks import make_identity

    f32 = mybir.dt.float32
    bf = mybir.dt.bfloat16
    f8 = mybir.dt.float8e4
    i16 = mybir.dt.int16

    nc = bacc.Bacc("TRN2", target_bir_lowering=False, debug=False,
                   num_devices=NCORES, num_swdge_queues=4)

    ms_d = nc.dram_tensor("ms", [P, nch1 * CHUNK_B * IN], bf, kind="ExternalInput")
    dst1_d = nc.dram_tensor("dst1", [P, nch1 * CHUNK_B], bf, kind="ExternalInput")
    idx2_d = [nc.dram_tensor(f"idx2_{g}", [P, nch2[g] * (NIDX // 16)], i16,
                             kind="ExternalInput") for g in range(NG)]
    dst2_d = [nc.dram_tensor(f"dst2_{g}", [P, nch2[g] * CHUNK_B], bf,
                             kind="ExternalInput") for g in range(NG)]
    dinvo_d = nc.dram_tensor("dinvo", [P, NWIN], f32, kind="ExternalInput")
    w1_d = nc.dram_tensor("w1", [P, IN // P, H], bf, kind="ExternalInput")
    w2_d = nc.dram_tensor("w2", [P, H // P, H], bf, kind="ExternalInput")
    wl_d = nc.dram_tensor("wl", [P, H // P, OUT], bf, kind="ExternalInput")
    b1_d = nc.dram_tensor("b1", [1, H], bf, kind="ExternalInput")
    b2_d = nc.dram_tensor("b2", [1, H], bf, kind="ExternalInput")
    bl_d = nc.dram_tensor("bl", [1, OUT], bf, kind="ExternalInput")
    out_d = nc.dram_tensor("out", [S_OWN, OUT], f32, kind="ExternalOutput")

    with tile.TileContext(nc) as tc, ExitStack() as ctx:
        cpool = ctx.enter_context(tc.tile_pool(name="const", bufs=1))
        dram = ctx.enter_context(tc.tile_pool(name="dram", bufs=1, space="DRAM"))
        l1pool = ctx.enter_context(tc.tile_pool(name="l1m", bufs=6))
        hopool = ctx.enter_context(tc.tile_pool(name="h1o", bufs=3))
        mpool = ctx.enter_context(tc.tile_pool(name="msg", bufs=10))
        spool = ctx.enter_context(tc.tile_pool(name="sel", bufs=5))
        ypool = ctx.enter_context(tc.tile_pool(name="ys", bufs=3))
        hpool = ctx.enter_context(tc.tile_pool(name="dense", bufs=3))
        psA = ctx.enter_context(tc.tile_pool(name="psA", bufs=2, space="PSUM"))
        psB = ctx.enter_context(tc.tile_pool(name="psB", bufs=2, space="PSUM"))
        psT = ctx.enter_context(tc.tile_pool(name="psT", bufs=2, space="PSUM"))

        # ---- constants
        iota_i = cpool.tile([P, P], mybir.dt.int32)
        iota_b = cpool.tile([P, CHUNK_B, P], bf)
        nc.gpsimd.iota(iota_i[:], pattern=[[1, P]], base=0, channel_multiplier=0)
        for bc in range(CHUNK_B):
            nc.vector.tensor_copy(out=iota_b[:, bc], in_=iota_i[:])
        ident = cpool.tile([P, P], bf)
        make_identity(nc, ident[:])
        ones_t = cpool.tile([1, P], bf)
        nc.vector.memset(ones_t[:], 1.0)

        dinvo = cpool.tile([P, NWIN], f32)
        nc.sync.dma_start(out=dinvo[:], in_=dinvo_d[:])

        dst1_t = cpool.tile([P, nch1, CHUNK_B], bf)
        nc.sync.dma_start(out=dst1_t[:], in_=dst1_d[:].rearrange(
            "p (c b) -> p c b", b=CHUNK_B))
        idx2_t = []
        dst2_t = []
        for g in range(NG):
            it = cpool.tile([P, nch2[g], NIDX // 16], mybir.dt.int16,
                            name=f"idx2t{g}")
            idx2_t.append(it)
            dt_ = cpool.tile([P, nch2[g], CHUNK_B], bf, name=f"dst2t{g}")
            dst2_t.append(dt_)

        def load_tables(g):
            # deferred: staggered through L1 so the startup burst does not
            # contend with the L1 stream loads
            nc.sync.dma_start(out=idx2_t[g][:], in_=idx2_d[g][:].rearrange(
                "p (c j) -> p c j", j=NIDX // 16))
            nc.sync.dma_start(out=dst2_t[g][:], in_=dst2_d[g][:].rearrange(
                "p (c b) -> p c b", b=CHUNK_B))

        w1_t = cpool.tile([P, IN // P, H], bf)
        w2_t = cpool.tile([P, H // P, H], bf)
        wl_t = cpool.tile([P, H // P, OUT], bf)
        b1_t = cpool.tile([1, H], bf)
        b2_t = cpool.tile([1, H], bf)
        bl_t = cpool.tile([1, OUT], bf)
        for t, d in ((w1_t, w1_d), (w2_t, w2_d), (wl_t, wl_d),
                     (b1_t, b1_d), (b2_t, b2_d), (bl_t, bl_d)):
            nc.sync.dma_start(out=t[:], in_=d[:])

        # accumulator for pipelined L2 (bf16; one column block per window)
        acc2 = cpool.tile([P, NWIN, H], bf)

        # ---- DRAM intermediates (per AllGather chunk so deps stay exact:
        # no false WAR between later L1 windows and an in-flight AllGather)
        gin2b = [dram.tile([RRE[g], H], bf, name=f"gin2b{g}")
                 for g in range(NG)]
        gin2f = [dram.tile([RRE[g], H], f8, name=f"gin2f{g}")
                 for g in range(NG)]
        gout = [dram.tile([8 * RRE[g], H], f8, addr_space="Shared",
                          name=f"gout{g}") for g in range(NG)]

        # ---- L1: host-materialized stream, window-major.  Loads are
        # super-chunks of SUP gather-chunks: partition-major layout gives
        # 16KB contiguous per partition per dma (big descriptors).
        loaded1s = {}
        loaded1 = {}
        SCB = SUP * CHUNK_B * IN

        def ensure1(c):
            if c in loaded1:
                return loaded1[c]
            s = c // SUP
            if s not in loaded1s:
                mts = l1pool.tile([P, SUP, CHUNK_B, IN], bf, tag="m1")
                # SWDGE: the gpsimd queue carries no per-window compute in L1,
                # so these loads issue as far ahead as the slots allow
                nc.gpsimd.dma_start(
                    out=mts[:], in_=ms_d[:, s * SCB : (s + 1) * SCB]
                    .rearrange("p (u b f) -> p u b f", u=SUP, b=CHUNK_B))
                loaded1s[s] = mts
            mt = loaded1s[s][:, c % SUP]
            st = spool.tile([P, CHUNK_B, P], bf, tag="sel")
            nc.vector.tensor_tensor(
                out=st[:], in0=iota_b[:],
                in1=dst1_t[:, c].to_broadcast([P, CHUNK_B, P]),
                op=mybir.AluOpType.is_equal)
            loaded1[c] = (mt, st)
            return mt, st

        TBL_AT = {2: 0, 14: 1, 28: 2, 40: 3}
        for w in range(NWIN):
            if w in TBL_AT:
                load_tables(TBL_AT[w])
            nrow = min(P, S_OWN - w * P)
            acc = psA.tile([P, IN], f32, tag="acc1", bufs=3)
            nbat = len(sched1[w])
            for i, (c, bc) in enumerate(sched1[w]):
                mt, st = ensure1(c)
                nc.tensor.matmul(out=acc[:], lhsT=st[:, bc], rhs=mt[:, bc],
                                 start=(i == 0), stop=(i == nbat - 1))
            ys = ypool.tile([P, IN], bf, tag="ys1")
            nc.scalar.mul(ys[:], acc[:], dinvo[:, w : w + 1])
            aggT = hpool.tile([P, IN // P, P], bf, tag="aggT1")
            for f in range(IN // P):
                tp = psT.tile([P, P], bf, tag="tp", bufs=1)
                nc.tensor.transpose(tp[:], ys[:, f * P : (f + 1) * P], ident[:])
                nc.scalar.copy(out=aggT[:, f], in_=tp[:])
            ph = psB.tile([P, H], f32, tag="mm")
            for f in range(IN // P):
                nc.tensor.matmul(out=ph[:], lhsT=aggT[:, f], rhs=w1_t[:, f],
                                 start=(f == 0), stop=False)
            nc.tensor.matmul(out=ph[:], lhsT=ones_t[:], rhs=b1_t[:],
                             start=False, stop=True)
            g2 = hpool.tile([P, H], bf, tag="g2")
            nc.scalar.activation(g2[:], ph[:], mybir.ActivationFunctionType.Relu,
                                 scale=dinvo[:, w : w + 1])
            g2f = hpool.tile([P, H], f8, tag="g2f")
            nc.scalar.activation(g2f[:], ph[:], mybir.ActivationFunctionType.Relu,
                                 scale=dinvo[:, w : w + 1])
            gw = next(g for g in range(NG) if w < CUMW[g + 1])
            wrow = (w - CUMW[gw]) * P
            nc.sync.dma_start(out=gin2b[gw][wrow : wrow + nrow], in_=g2[:nrow])
            nc.sync.dma_start(out=gin2f[gw][wrow : wrow + nrow], in_=g2f[:nrow])

            for g in range(NG):
                if w == CUMW[g + 1] - 1:
                    nc.gpsimd.collective_compute(
                        "AllGather", mybir.AluOpType.bypass,
                        replica_groups=[list(range(NCORES))],
                        ins=[gin2f[g][:]],
                        outs=[gout[g][:]])

        # ---- L2: chunk-major pipelined aggregation
        rrq = [0]

        def ensure2(g, c, loaded2):
            if (g, c) in loaded2:
                return loaded2[(g, c)]
            mt = mpool.tile([P, CHUNK_B, H], f8, tag="m2")
            nc.gpsimd.dma_gather(
                out_ap=mt[:], in_ap=gout[g][:],
                idxs_ap=idx2_t[g][:, c], num_idxs=NIDX, num_idxs_reg=NIDX,
                elem_size=H, queue_num=rrq[0])
            rrq[0] = (rrq[0] + 1) % 4
            st = spool.tile([P, CHUNK_B, P], f8, tag="sel2")
            nc.vector.tensor_tensor(
                out=st[:], in0=iota_b[:],
                in1=dst2_t[g][:, c].to_broadcast([P, CHUNK_B, P]),
                op=mybir.AluOpType.is_equal)
            loaded2[(g, c)] = (mt, st)
            return mt, st

        def tail2(w, src):
            nrow = min(P, S_OWN - w * P)
            ys2 = ypool.tile([P, H], bf, tag="ys2")
            nc.scalar.mul(ys2[:], src, dinvo[:, w : w + 1])
            aggT = hpool.tile([P, H // P, P], bf, tag="aggT2")
            for f in range(H // P):
                tp = psT.tile([P, P], bf, tag="tp", bufs=1)
                nc.tensor.transpose(tp[:], ys2[:, f * P : (f + 1) * P], ident[:])
                nc.scalar.copy(out=aggT[:, f], in_=tp[:])
            # h2T computed directly in transposed form: no second transpose
            ph2T = psB.tile([P, H], f32, tag="mm")
            for fh in range(H // P):
                o = ph2T[:, fh * P : (fh + 1) * P]
                for fc in range(H // P):
                    nc.tensor.matmul(out=o, rhs=aggT[:, fc],
                                     lhsT=w2_t[:, fc, fh * P : (fh + 1) * P],
                                     start=(fc == 0), stop=False)
                nc.tensor.matmul(out=o, lhsT=b2_t[:, fh * P : (fh + 1) * P],
                                 rhs=ones_t[:], start=False, stop=True)
            h2T = hpool.tile([P, H // P, P], bf, tag="h2T")
            nc.scalar.activation(h2T[:],
                                 ph2T[:].rearrange("p (f j) -> p f j", j=P),
                                 mybir.ActivationFunctionType.Relu)
            ph3 = psB.tile([P, OUT], f32, tag="mm")
            for f in range(H // P):
                nc.tensor.matmul(out=ph3[:], lhsT=h2T[:, f], rhs=wl_t[:, f],
                                 start=(f == 0), stop=False)
            nc.tensor.matmul(out=ph3[:], lhsT=ones_t[:], rhs=bl_t[:],
                             start=False, stop=True)
            ot = hpool.tile([P, OUT], f32, tag="ot")
            nc.scalar.copy(out=ot[:], in_=ph3[:])
            nc.sync.dma_start(out=out_d[w * P : w * P + nrow], in_=ot[:nrow])

        # chunk passes: batches of chunks {0,1}, {2,3}, {4} accumulate in one
        # PSUM group per window per pass (fewer preloads + acc2 roundings)
        PASSES = [[0, 1], [2, 3]]
        loaded2 = {}
        dr = mybir.MatmulPerfMode.DoubleRow
        for pi, gs in enumerate(PASSES):
            last_pass = pi == len(PASSES) - 1
            for w in range(NWIN):
                items = [(g, c, bc) for g in gs for (c, bc) in sched2[g][w]]
                pp = psA.tile([P, H], f32, tag="acc2")
                if pi == 0:
                    # self-loop: identity matmul of own (bf16) h1 rows
                    h1own = hopool.tile([P, H], bf, tag="h1own")
                    nrow = min(P, S_OWN - w * P)
                    gw = next(gg for gg in range(NG) if w < CUMW[gg + 1])
                    wrow = (w - CUMW[gw]) * P
                    if nrow < P:
                        nc.vector.memset(h1own[:], 0.0)
                    nc.sync.dma_start(out=h1own[:nrow],
                                      in_=gin2b[gw][wrow : wrow + nrow])
                    nc.tensor.matmul(out=pp[:], lhsT=ident[:], rhs=h1own[:],
                                     start=True, stop=(len(items) == 0))
                else:
                    # resume accumulation: preload prior partial sum via PE
                    nc.tensor.matmul(out=pp[:], lhsT=ident[:], rhs=acc2[:, w],
                                     start=True, stop=(len(items) == 0))
                i = 0
                n = len(items)
                while i < n:
                    g, c, bc = items[i]
                    mt, st = ensure2(g, c, loaded2)
                    if (i + 1 < n and items[i + 1] == (g, c, bc + 1)):
                        # fp8 DoubleRow: two 128-edge batches per instruction
                        nc.tensor.matmul(out=pp[:], lhsT=st[:, bc : bc + 2],
                                         rhs=mt[:, bc : bc + 2], start=False,
                                         stop=(i + 2 == n), perf_mode=dr)
                        i += 2
                    else:
                        nc.tensor.matmul(out=pp[:], lhsT=st[:, bc],
                                         rhs=mt[:, bc], start=False,
                                         stop=(i + 1 == n))
                        i += 1
                if not last_pass:
                    nc.scalar.copy(out=acc2[:, w], in_=pp[:])
                else:
                    tail2(w, pp[:])

    nc.compile()
    return nc


_CACHE = {}


def _make_in_maps(inputs, prep):
    x = np.asarray(inputs["x"], dtype=np.float32)
    W1 = np.asarray(inputs["W1"], dtype=np.float32)
    b1 = np.asarray(inputs["b1"], dtype=np.float32)
    W2 = np.asarray(inputs["W2"], dtype=np.float32)
    b2 = np.asarray(inputs["b2"], dtype=np.float32)
    Wl = np.asarray(inputs["Wl"], dtype=np.float32)
    bl = np.asarray(inputs["bl"], dtype=np.float32)

    dinv = prep["dinv"]
    xs = (x * dinv[:, None]).astype(BF16)         # pre-scaled source features

    w1b = W1.reshape(IN // P, P, H).transpose(1, 0, 2).astype(BF16)
    w2b = W2.reshape(H // P, P, H).transpose(1, 0, 2).astype(BF16)
    wlb = Wl.reshape(H // P, P, OUT).transpose(1, 0, 2).astype(BF16)

    nch1 = prep["nch1"]
    in_maps = []
    for k in range(NCORES):
        # L1 message stream, partition-major: [P, nch1*CHUNK_B*IN]; edge p
        # of batch (c,bc) lands at [p, (c*CHUNK_B+bc)*IN :]
        ms = xs[prep["src1"][k]]                  # [nch1*NIDX, IN]
        ms = ms.reshape(nch1, CHUNK_B, P, IN).transpose(2, 0, 1, 3)
        ms = np.ascontiguousarray(ms).reshape(P, nch1 * CHUNK_B * IN)
        dv = np.ones(NWIN * P, dtype=np.float32)
        dv[:S_OWN] = dinv[k * S_OWN : (k + 1) * S_OWN]
        dv = np.ascontiguousarray(dv.reshape(NWIN, P).T)
        im = {
            "ms": ms,
            "dst1": prep["dst1_t"][k].reshape(P, -1).astype(BF16),
            "dinvo": dv,
            "w1": w1b, "w2": w2b, "wl": wlb,
            "b1": b1.reshape(1, H).astype(BF16),
            "b2": b2.reshape(1, H).astype(BF16),
            "bl": bl.reshape(1, OUT).astype(BF16),
        }
        for g in range(NG):
            im[f"idx2_{g}"] = prep["idx2_t"][g][k].reshape(P, -1)
            im[f"dst2_{g}"] = prep["dst2_t"][g][k].reshape(P, -1).astype(BF16)
        in_maps.append(im)
    return in_maps


def kernel(**inputs):
    from concourse.bass_utils import run_bass_kernel_spmd

    edge_index = np.asarray(inputs["edge_index"])
    prep = _prep(edge_index)
    key = (prep["nch1"], tuple(prep["nch2"]))
    if key not in _CACHE:
        _CACHE[key] = _build_nc(prep["nch1"], prep["nch2"],
                                prep["sched1"], prep["sched2"])
    nc = _CACHE[key]
    global _LAST_NC
    _LAST_NC = nc.m
    in_maps = _make_in_maps(inputs, prep)

    res = run_bass_kernel_spmd(nc, in_maps, core_ids=list(range(NCORES)))
    out = np.concatenate([res.results[k]["out"] for k in range(NCORES)], axis=0)
    return out
